# revision 33
# baseline (speedup 1.0000x reference)
"""Trainium2 Bass kernel for nn_AGCRNCellWithMLP (AGCRN cell with per-node MLP weights).

Math (with nodes_ind == arange(N), which the harness guarantees):
    xh       = concat([x, h], -1)                      # [N, 129]
    combined = adj @ xh                                # [N, 129]
    r = sigmoid(mlp(combined, q, W_r, b_r))            # [N, 64]
    u = sigmoid(mlp(combined, q, W_u, b_u))
    h2 = r * h
    cand = tanh(mlp(concat([x, h2], -1), q, W_c, b_c))
    out = (1 - u) * h2 + u * cand
where mlp(v, q, W, b)[n, o] = sum_{d,i} q[n,d] v[n,i] W[d,i,o] + (q @ b)[n, o].

Sharding: data-parallel over nodes, 512 rows per core x 8 cores, fully
independent per core (no collectives); host replicates x/h and pre-transposes
per-core slices. All matmul tensors are float32r (fp32 storage, fast PE mode;
needs moving dim >= 256), so every matmul here uses N=256 node-columns.

Per-core pipeline, run twice (once per 256-node half, pipelined):
  1. combT[129, 256] = (adj_slice @ xh)^T directly: lhsT = xh k-tiles
     ([128,128] + [128,1] column splits), rhs = adjT k-slabs [128, 256].
     The 129th feature row accumulates in spare columns of the same PSUM bank.
  2. Z^T[r=(i*16+d), n] = V[i, n] * qT[d, n] for V in {combT, xh2T}, built 128
     rows per tile: a = sel_t.T @ V (PE replication matmul), z = a * q_rep (DVE).
  3. Gate preact G^T[64, 256] = b_g^T qT + sum_t Wf_g[t]^T Z^T[t] in PSUM;
     sigmoid/tanh on ScalarE; h2/output elementwise on DVE; DMA out f32.
"""
import sys

sys.path.insert(0, "/opt/trn_rl_repo")

import numpy as np

N = 4096
IN = 64
QD = 16
CI = 2 * IN + 1          # 129
NCORES = 8
NS = N // NCORES         # 512 nodes per core
NH = NS // 2             # 256 nodes per half
KT = N // 128            # 32 k-tiles for the adj matmul
RT = 17                  # ceil(CI*QD/128) z-tiles (2176 padded rows)
RPAD = RT * 128          # 2176
CI2 = CI + 1             # xh slab width: 129 + duplicated last col

_CACHE = {}


def build_nc():
    import concourse.bass as bass
    import concourse.bacc as bacc
    import concourse.tile as tile
    import concourse.mybir as mybir

    F32 = mybir.dt.float32
    F32R = mybir.dt.float32r
    ACT = mybir.ActivationFunctionType

    nc = bacc.Bacc()
    dp = nc.declare_dram_parameter
    adjT_e = dp("adjT", [128, 2 * KT * NH], F32R, isOutput=False)  # SBUF-layout [p, (half,ktile,n)]
    xh_e = dp("xh", [128, KT * CI2], F32R, isOutput=False)    # SBUF-layout [p, (ktile,f)]
    qT_e = dp("qT", [QD, NS], F32R, isOutput=False)
    qrep_e = dp("qrep", [128, 2 * NS], F32, isOutput=False)   # qT tiled x8 down partitions, x2 along free
    xT_e = dp("xT", [IN + 2, NS], F32R, isOutput=False)
    hT_e = dp("hT", [IN, NS], F32, isOutput=False)
    wfru_e = dp("wfru", [128, RT * 2 * IN], F32R, isOutput=False)
    wfc_e = dp("wfc", [128, RT * IN], F32R, isOutput=False)
    bru_e = dp("bru", [QD, 2 * IN], F32R, isOutput=False)
    bc_e = dp("bc", [QD, IN], F32R, isOutput=False)
    sel_e = dp("sel", [128, 16 * 128], F32R, isOutput=False)
    sel16_e = dp("sel16", [2, 128], F32R, isOutput=False)
    out_e = dp("out", [IN, NS], F32, isOutput=True)

    with tile.TileContext(nc) as tc:
        with tc.tile_pool(name="const", bufs=1) as cpool, \
             tc.tile_pool(name="big", bufs=1) as bigpool, \
             tc.tile_pool(name="half", bufs=2) as hpool, \
             tc.tile_pool(name="zt", bufs=6) as ztpool, \
             tc.tile_pool(name="psC", bufs=2, space="PSUM") as psC, \
             tc.tile_pool(name="psZ", bufs=2, space="PSUM") as psZ, \
             tc.tile_pool(name="psG", bufs=2, space="PSUM") as psG:

            # ---- static loads (emission order ~= DMA service order) -----------
            # xh first (first matmul needs it), then adjT half 0, then the
            # small gate constants, then adjT half 1.
            xh = bigpool.tile([128, KT * CI2], F32R)
            adjT = bigpool.tile([128, 2 * KT * NH], F32R)

            def load_adjT_chunk(h, ch, nch=4):
                w = KT // nch
                lo = (h * KT + ch * w) * NH
                hi = (h * KT + (ch + 1) * w) * NH
                nc.sync.dma_start(adjT[:, lo:hi], adjT_e[:, lo:hi])

            # adjT half-0 in ramped chunks so the first matmul starts ASAP;
            # gate constants ordered by first-use time; adjT half-1 interleaved
            # so h1 adj matmuls can fill PE gaps during h0's gate phases.
            adj_off = [0]

            def load_adjT_slabs(h, nslab):
                lo = (h * KT + adj_off[0]) * NH
                hi = (h * KT + adj_off[0] + nslab) * NH
                nc.sync.dma_start(adjT[:, lo:hi], adjT_e[:, lo:hi])
                adj_off[0] = (adj_off[0] + nslab) % KT

            xh_off = [0]

            def load_xh_slabs(nslab):
                xlo = xh_off[0] * CI2
                xhi = (xh_off[0] + nslab) * CI2
                nc.sync.dma_start(xh[:, xlo:xhi], xh_e[:, xlo:xhi])
                xh_off[0] += nslab

            load_xh_slabs(4)
            for nslab in (1, 1, 2, 4):
                load_adjT_slabs(0, nslab)
            load_xh_slabs(4)
            load_adjT_slabs(0, 8)
            load_xh_slabs(8)
            load_adjT_slabs(0, 8)
            load_xh_slabs(16)
            load_adjT_slabs(0, 8)
            wfru = cpool.tile([128, RT * 2 * IN], F32R, tag="wfru")
            nc.sync.dma_start(wfru[:], wfru_e[:])
            sel = cpool.tile([128, 16 * 128], F32R, tag="sel")
            nc.sync.dma_start(sel[:], sel_e[:])
            sel16 = cpool.tile([2, 128], F32R, tag="sel16")
            nc.sync.dma_start(sel16[:], sel16_e[:])
            qT = cpool.tile([QD, NS], F32R, tag="qT")
            nc.sync.dma_start(qT[:], qT_e[:])
            qrep = cpool.tile([128, 2, NS], F32, tag="qrep")
            nc.sync.dma_start(qrep[:], qrep_e[:])
            bru = cpool.tile([QD, 2 * IN], F32R, tag="bru")
            nc.sync.dma_start(bru[:], bru_e[:])
            wfc = cpool.tile([128, RT * IN], F32R, tag="wfc")
            nc.sync.dma_start(wfc[:], wfc_e[:])
            bc = cpool.tile([QD, IN], F32R, tag="bc")
            nc.sync.dma_start(bc[:], bc_e[:])
            load_adjT_slabs(1, 8)
            load_adjT_slabs(1, 8)
            hT = cpool.tile([IN, NS], F32, tag="hT")
            nc.sync.dma_start(hT[:], hT_e[:])
            xT = cpool.tile([IN + 2, NS], F32R, tag="xT")
            nc.sync.dma_start(xT[:], xT_e[:])
            load_adjT_slabs(1, 8)
            load_adjT_slabs(1, 8)

            for h in range(2):
                cols = slice(h * NH, (h + 1) * NH)

                # ---- 1. combT = (adj_h @ xh)^T --------------------------------
                pc = psC.tile([128, NH], F32, tag="pc", name=f"pc{h}")
                pl = psC.tile([2, NH], F32, tag="pl", name=f"pl{h}")
                for t in range(KT):
                    rhs = adjT[:, (h * KT + t) * NH:(h * KT + t + 1) * NH]
                    nc.tensor.matmul(pc[:], xh[:, t * CI2: t * CI2 + 128], rhs,
                                     start=(t == 0), stop=(t == KT - 1))
                    nc.tensor.matmul(pl[:], xh[:, t * CI2 + 128: t * CI2 + 130], rhs,
                                     start=(t == 0), stop=(t == KT - 1))
                combT = hpool.tile([128, NH], F32R, tag="combT", name=f"combT{h}")
                nc.vector.tensor_copy(combT[:], pc[:])
                combTt = hpool.tile([2, NH], F32R, tag="combTt", name=f"combTt{h}")
                nc.vector.tensor_copy(combTt[:], pl[:])

                # ---- 2+3. gates -----------------------------------------------
                def gate_pass(Vm, Vt, wft, bt, ps_g, mw, tag):
                    nc.tensor.matmul(ps_g[:], bt[:], qT[:, cols],
                                     start=True, stop=False)
                    for t in range(RT):
                        az = psZ.tile([128, NH], F32, tag="az", name=f"az{tag}{h}_{t}")
                        if t < 16:
                            nc.tensor.matmul(az[:], sel[:, t * 128:(t + 1) * 128],
                                             Vm[:], start=True, stop=True)
                        else:
                            nc.tensor.matmul(az[:], sel16[:], Vt[:],
                                             start=True, stop=True)
                        z = ztpool.tile([128, NH], F32R, tag="z")
                        nc.vector.tensor_mul(z[:], az[:], qrep[:, 0, cols])
                        nc.tensor.matmul(ps_g[:], wft[:, t * mw:(t + 1) * mw], z[:],
                                         start=False, stop=(t == RT - 1))

                ps_ru = psG.tile([2 * IN, NH], F32, tag="g", name=f"gru{h}")
                gate_pass(combT, combTt, wfru, bru, ps_ru, 2 * IN, "ru")
                r_sb = hpool.tile([IN, NH], F32, tag="r_sb", name=f"r{h}")
                nc.scalar.activation(r_sb[:], ps_ru[0:IN, :], ACT.Sigmoid)
                u_sb = hpool.tile([IN, NH], F32, tag="u_sb", name=f"u{h}")
                nc.scalar.activation(u_sb[:], ps_ru[IN:2 * IN, :], ACT.Sigmoid)

                # xh2T rows: 0..63 = h2T, 64..127 = x features 0..63,
                # tail row 0 = x feature 64 (Wf_c rows permuted to match).
                xh2T = hpool.tile([128, NH], F32R, tag="xh2T", name=f"xh2T{h}")
                xh2Tt = hpool.tile([2, NH], F32R, tag="xh2Tt", name=f"xh2Tt{h}")
                nc.vector.tensor_mul(xh2T[0:IN, :], r_sb[:], hT[:, cols])
                nc.vector.tensor_copy(xh2T[IN:128, :], xT[0:IN, cols])
                nc.vector.tensor_copy(xh2Tt[:], xT[IN:IN + 2, cols])

                ps_c2 = psG.tile([IN, NH], F32, tag="g", name=f"gc{h}")
                gate_pass(xh2T, xh2Tt, wfc, bc, ps_c2, IN, "c")
                cand = hpool.tile([IN, NH], F32, tag="cand", name=f"cand{h}")
                nc.scalar.activation(cand[:], ps_c2[:], ACT.Tanh)

                # ---- 4. out = h2 + u * (cand - h2) ----------------------------
                dt_ = hpool.tile([IN, NH], F32, tag="dt", name=f"dt{h}")
                nc.vector.tensor_sub(dt_[:], cand[:], xh2T[0:IN, :])
                et = hpool.tile([IN, NH], F32, tag="et", name=f"et{h}")
                nc.vector.tensor_mul(et[:], u_sb[:], dt_[:])
                outT = hpool.tile([IN, NH], F32, tag="outT", name=f"outT{h}")
                nc.vector.tensor_add(outT[:], xh2T[0:IN, :], et[:])
                nc.sync.dma_start(out_e[:, cols], outT[:])
    nc.compile()
    return nc


def _f32(a):
    return np.ascontiguousarray(np.asarray(a, np.float32))


def prep_in_maps(x, h, query_vectors, adj, nodes_ind, W_r, b_r, W_u, b_u, W_c, b_c):
    x = _f32(x)
    h = _f32(h)
    q = _f32(query_vectors)
    adj = np.asarray(adj, np.float32)
    ni = np.asarray(nodes_ind)
    assert np.array_equal(ni, np.arange(N)), "kernel assumes nodes_ind == arange(N)"

    xh = np.concatenate([x, h, h[:, -1:]], axis=-1)           # [N, 130] (last col 2x)
    xh_sb = _f32(xh.reshape(KT, 128, CI2).transpose(1, 0, 2).reshape(128, KT * CI2))
    wfs, bs = {}, {}
    # xh2 feature order for gate c: [h2(0..63), x(0..63), x(64)]
    perm_c = list(range(IN + 1, CI)) + list(range(0, IN)) + [IN]
    for g, W, b in (("r", W_r, b_r), ("u", W_u, b_u), ("c", W_c, b_c)):
        Wt = np.asarray(W, np.float32).transpose(1, 0, 2)     # [129(i), 16, 64]
        if g == "c":
            Wt = Wt[perm_c]
        Wim = Wt.reshape(CI * QD, IN)
        Wp = np.concatenate(
            [Wim, np.zeros((RPAD - CI * QD, IN), np.float32)], axis=0)
        wfs[g] = Wp.reshape(RT, 128, IN)
        bs[g] = np.asarray(b, np.float32)

    wfru = _f32(np.concatenate([wfs["r"], wfs["u"]], axis=2)
                .transpose(1, 0, 2).reshape(128, RT * 2 * IN))
    wfc = _f32(wfs["c"].transpose(1, 0, 2).reshape(128, RT * IN))
    bru = _f32(np.concatenate([bs["r"], bs["u"]], axis=1))
    bc = _f32(bs["c"])

    sel = np.zeros((128, 16 * 128), np.float32)
    for t in range(16):
        for p in range(128):
            sel[8 * t + p // 16, t * 128 + p] = 1.0
    sel16 = np.zeros((2, 128), np.float32)
    sel16[0, 0:16] = 1.0

    in_maps = []
    for c in range(NCORES):
        s = slice(c * NS, (c + 1) * NS)
        qTc = _f32(q[s].T)                                    # [16, 512]
        # adjT SBUF layout [p, (half, ktile, n)]: adjT[k, n] = adj[s][n, k]
        a = adj[s].T.reshape(KT, 128, 2, NH)                  # [t, p, half, n]
        adjT_sb = _f32(a.transpose(1, 2, 0, 3).reshape(128, 2 * KT * NH))
        in_maps.append({
            "adjT": adjT_sb,
            "xh": xh_sb,
            "qT": qTc,
            "qrep": np.ascontiguousarray(np.tile(qTc, (2 * (128 // QD), 1)).reshape(2, 128, NS).transpose(1, 0, 2).reshape(128, 2 * NS)),
            "xT": _f32(np.concatenate([x[s].T, np.zeros((1, NS), np.float32)], axis=0)),
            "hT": _f32(h[s].T),
            "wfru": wfru, "wfc": wfc, "bru": bru, "bc": bc,
            "sel": sel, "sel16": sel16,
        })
    return in_maps


def kernel(**inputs):
    from concourse.bass_utils import run_bass_kernel_spmd

    if "nc" not in _CACHE:
        _CACHE["nc"] = build_nc()
    nc = _CACHE["nc"]
    in_maps = prep_in_maps(**inputs)
    res = run_bass_kernel_spmd(nc, in_maps, core_ids=list(range(NCORES)))
    out = np.empty((N, IN), np.float32)
    for c in range(NCORES):
        out[c * NS:(c + 1) * NS, :] = res.results[c]["out"].T
    return out


# revision 34
# speedup vs baseline: 1.0684x; 1.0684x over previous
"""Trainium2 Bass kernel for nn_AGCRNCellWithMLP (AGCRN cell with per-node MLP weights).

Math (with nodes_ind == arange(N), which the harness guarantees):
    xh       = concat([x, h], -1)                      # [N, 129]
    combined = adj @ xh                                # [N, 129]
    r = sigmoid(mlp(combined, q, W_r, b_r))            # [N, 64]
    u = sigmoid(mlp(combined, q, W_u, b_u))
    h2 = r * h
    cand = tanh(mlp(concat([x, h2], -1), q, W_c, b_c))
    out = (1 - u) * h2 + u * cand
where mlp(v, q, W, b)[n, o] = sum_{d,i} q[n,d] v[n,i] W[d,i,o] + (q @ b)[n, o].

Sharding: data-parallel over nodes, 512 rows per core x 8 cores, fully
independent per core (no collectives); host replicates x/h and pre-transposes
per-core slices. All matmul tensors are float32r (fp32 storage, fast PE mode;
needs moving dim >= 256), so every matmul here uses N=256 node-columns.

Per-core pipeline, run twice (once per 256-node half, pipelined):
  1. combT[129, 256] = (adj_slice @ xh)^T directly: lhsT = xh k-tiles
     ([128,128] + [128,1] column splits), rhs = adjT k-slabs [128, 256].
     The 129th feature row accumulates in spare columns of the same PSUM bank.
  2. Z^T[r=(i*16+d), n] = V[i, n] * qT[d, n] for V in {combT, xh2T}, built 128
     rows per tile: a = sel_t.T @ V (PE replication matmul), z = a * q_rep (DVE).
  3. Gate preact G^T[64, 256] = b_g^T qT + sum_t Wf_g[t]^T Z^T[t] in PSUM;
     sigmoid/tanh on ScalarE; h2/output elementwise on DVE; DMA out f32.
"""
import sys

sys.path.insert(0, "/opt/trn_rl_repo")

import numpy as np

N = 4096
IN = 64
QD = 16
CI = 2 * IN + 1          # 129
NCORES = 8
NS = N // NCORES         # 512 nodes per core
NH = NS // 2             # 256 nodes per half
KT = N // 128            # 32 k-tiles for the adj matmul
RT = 17                  # ceil(CI*QD/128) z-tiles (2176 padded rows)
RPAD = RT * 128          # 2176
CI2 = CI + 1             # xh slab width: 129 + duplicated last col

_CACHE = {}


def build_nc():
    import concourse.bass as bass
    import concourse.bacc as bacc
    import concourse.tile as tile
    import concourse.mybir as mybir

    F32 = mybir.dt.float32
    F32R = mybir.dt.float32r
    ACT = mybir.ActivationFunctionType

    nc = bacc.Bacc()
    dp = nc.declare_dram_parameter
    adjT_e = dp("adjT", [128, 2 * KT * NH], F32R, isOutput=False)  # SBUF-layout [p, (half,ktile,n)]
    xh_e = dp("xh", [128, KT * CI2], F32R, isOutput=False)    # SBUF-layout [p, (ktile,f)]
    qT_e = dp("qT", [QD, NS], F32R, isOutput=False)
    qrep_e = dp("qrep", [128, 2 * NS], F32, isOutput=False)   # qT tiled x8 down partitions, x2 along free
    xT_e = dp("xT", [IN + 2, NS], F32R, isOutput=False)
    hT_e = dp("hT", [IN, NS], F32, isOutput=False)
    wfru_e = dp("wfru", [128, RT * 2 * IN], F32R, isOutput=False)
    wfc_e = dp("wfc", [128, RT * IN], F32R, isOutput=False)
    bru_e = dp("bru", [QD, 2 * IN], F32R, isOutput=False)
    bc_e = dp("bc", [QD, IN], F32R, isOutput=False)
    sel_e = dp("sel", [128, 16 * 128], F32R, isOutput=False)
    sel16_e = dp("sel16", [2, 128], F32R, isOutput=False)
    out_e = dp("out", [IN, NS], F32, isOutput=True)

    with tile.TileContext(nc) as tc:
        with tc.tile_pool(name="const", bufs=1) as cpool, \
             tc.tile_pool(name="big", bufs=1) as bigpool, \
             tc.tile_pool(name="half", bufs=2) as hpool, \
             tc.tile_pool(name="zt", bufs=6) as ztpool, \
             tc.tile_pool(name="psC", bufs=2, space="PSUM") as psC, \
             tc.tile_pool(name="psZ", bufs=2, space="PSUM") as psZ, \
             tc.tile_pool(name="psG", bufs=2, space="PSUM") as psG:

            # ---- static loads (emission order ~= DMA service order) -----------
            # xh first (first matmul needs it), then adjT half 0, then the
            # small gate constants, then adjT half 1.
            xh = bigpool.tile([128, KT * CI2], F32R)
            adjT = bigpool.tile([128, 2 * KT * NH], F32R)

            def load_adjT_chunk(h, ch, nch=4):
                w = KT // nch
                lo = (h * KT + ch * w) * NH
                hi = (h * KT + (ch + 1) * w) * NH
                nc.sync.dma_start(adjT[:, lo:hi], adjT_e[:, lo:hi])

            # adjT half-0 in ramped chunks so the first matmul starts ASAP;
            # gate constants ordered by first-use time; adjT half-1 interleaved
            # so h1 adj matmuls can fill PE gaps during h0's gate phases.
            adj_off = [0]

            def load_adjT_slabs(h, nslab):
                lo = (h * KT + adj_off[0]) * NH
                hi = (h * KT + adj_off[0] + nslab) * NH
                nc.sync.dma_start(adjT[:, lo:hi], adjT_e[:, lo:hi])
                adj_off[0] = (adj_off[0] + nslab) % KT

            xh_off = [0]

            def load_xh_slabs(nslab):
                xlo = xh_off[0] * CI2
                xhi = (xh_off[0] + nslab) * CI2
                nc.sync.dma_start(xh[:, xlo:xhi], xh_e[:, xlo:xhi])
                xh_off[0] += nslab

            load_xh_slabs(4)
            for nslab in (1, 1, 2, 4):
                load_adjT_slabs(0, nslab)
            load_xh_slabs(4)
            load_adjT_slabs(0, 8)
            load_xh_slabs(8)
            load_adjT_slabs(0, 8)
            load_xh_slabs(16)
            load_adjT_slabs(0, 8)
            wfru = cpool.tile([128, RT * 2 * IN], F32R, tag="wfru")
            nc.sync.dma_start(wfru[:], wfru_e[:])
            sel = cpool.tile([128, 16 * 128], F32R, tag="sel")
            nc.sync.dma_start(sel[:], sel_e[:])
            sel16 = cpool.tile([2, 128], F32R, tag="sel16")
            nc.sync.dma_start(sel16[:], sel16_e[:])
            qT = cpool.tile([QD, NS], F32R, tag="qT")
            nc.sync.dma_start(qT[:], qT_e[:])
            qrep = cpool.tile([128, 2, NS], F32, tag="qrep")
            nc.sync.dma_start(qrep[:], qrep_e[:])
            bru = cpool.tile([QD, 2 * IN], F32R, tag="bru")
            nc.sync.dma_start(bru[:], bru_e[:])
            wfc = cpool.tile([128, RT * IN], F32R, tag="wfc")
            nc.sync.dma_start(wfc[:], wfc_e[:])
            bc = cpool.tile([QD, IN], F32R, tag="bc")
            nc.sync.dma_start(bc[:], bc_e[:])
            hT = cpool.tile([IN, NS], F32, tag="hT")
            nc.sync.dma_start(hT[:], hT_e[:])
            xT = cpool.tile([IN + 2, NS], F32R, tag="xT")
            nc.sync.dma_start(xT[:], xT_e[:])
            for _ in range(4):
                load_adjT_slabs(1, 8)

            for h in range(2):
                cols = slice(h * NH, (h + 1) * NH)

                # ---- 1. combT = (adj_h @ xh)^T --------------------------------
                pc = psC.tile([128, NH], F32, tag="pc", name=f"pc{h}")
                pl = psC.tile([2, NH], F32, tag="pl", name=f"pl{h}")
                for t in range(KT):
                    rhs = adjT[:, (h * KT + t) * NH:(h * KT + t + 1) * NH]
                    nc.tensor.matmul(pc[:], xh[:, t * CI2: t * CI2 + 128], rhs,
                                     start=(t == 0), stop=(t == KT - 1))
                    nc.tensor.matmul(pl[:], xh[:, t * CI2 + 128: t * CI2 + 130], rhs,
                                     start=(t == 0), stop=(t == KT - 1))
                combT = hpool.tile([128, NH], F32R, tag="combT", name=f"combT{h}")
                nc.vector.tensor_copy(combT[:], pc[:])
                combTt = hpool.tile([2, NH], F32R, tag="combTt", name=f"combTt{h}")
                nc.vector.tensor_copy(combTt[:], pl[:])

                # ---- 2+3. gates -----------------------------------------------
                def gate_pass(Vm, Vt, wft, bt, ps_g, mw, tag):
                    nc.tensor.matmul(ps_g[:], bt[:], qT[:, cols],
                                     start=True, stop=False)
                    for t in range(RT):
                        az = psZ.tile([128, NH], F32, tag="az", name=f"az{tag}{h}_{t}")
                        if t < 16:
                            nc.tensor.matmul(az[:], sel[:, t * 128:(t + 1) * 128],
                                             Vm[:], start=True, stop=True)
                        else:
                            nc.tensor.matmul(az[:], sel16[:], Vt[:],
                                             start=True, stop=True)
                        z = ztpool.tile([128, NH], F32R, tag="z")
                        nc.vector.tensor_mul(z[:], az[:], qrep[:, 0, cols])
                        nc.tensor.matmul(ps_g[:], wft[:, t * mw:(t + 1) * mw], z[:],
                                         start=False, stop=(t == RT - 1))

                ps_ru = psG.tile([2 * IN, NH], F32, tag="g", name=f"gru{h}")
                gate_pass(combT, combTt, wfru, bru, ps_ru, 2 * IN, "ru")
                r_sb = hpool.tile([IN, NH], F32, tag="r_sb", name=f"r{h}")
                nc.scalar.activation(r_sb[:], ps_ru[0:IN, :], ACT.Sigmoid)
                u_sb = hpool.tile([IN, NH], F32, tag="u_sb", name=f"u{h}")
                nc.scalar.activation(u_sb[:], ps_ru[IN:2 * IN, :], ACT.Sigmoid)

                # xh2T rows: 0..63 = h2T, 64..127 = x features 0..63,
                # tail row 0 = x feature 64 (Wf_c rows permuted to match).
                xh2T = hpool.tile([128, NH], F32R, tag="xh2T", name=f"xh2T{h}")
                xh2Tt = hpool.tile([2, NH], F32R, tag="xh2Tt", name=f"xh2Tt{h}")
                nc.vector.tensor_mul(xh2T[0:IN, :], r_sb[:], hT[:, cols])
                nc.vector.tensor_copy(xh2T[IN:128, :], xT[0:IN, cols])
                nc.vector.tensor_copy(xh2Tt[:], xT[IN:IN + 2, cols])

                ps_c2 = psG.tile([IN, NH], F32, tag="g", name=f"gc{h}")
                gate_pass(xh2T, xh2Tt, wfc, bc, ps_c2, IN, "c")
                cand = hpool.tile([IN, NH], F32, tag="cand", name=f"cand{h}")
                nc.scalar.activation(cand[:], ps_c2[:], ACT.Tanh)

                # ---- 4. out = h2 + u * (cand - h2) ----------------------------
                dt_ = hpool.tile([IN, NH], F32, tag="dt", name=f"dt{h}")
                nc.vector.tensor_sub(dt_[:], cand[:], xh2T[0:IN, :])
                et = hpool.tile([IN, NH], F32, tag="et", name=f"et{h}")
                nc.vector.tensor_mul(et[:], u_sb[:], dt_[:])
                outT = hpool.tile([IN, NH], F32, tag="outT", name=f"outT{h}")
                nc.vector.tensor_add(outT[:], xh2T[0:IN, :], et[:])
                nc.sync.dma_start(out_e[:, cols], outT[:])
    nc.compile()
    return nc


def _f32(a):
    return np.ascontiguousarray(np.asarray(a, np.float32))


def prep_in_maps(x, h, query_vectors, adj, nodes_ind, W_r, b_r, W_u, b_u, W_c, b_c):
    x = _f32(x)
    h = _f32(h)
    q = _f32(query_vectors)
    adj = np.asarray(adj, np.float32)
    ni = np.asarray(nodes_ind)
    assert np.array_equal(ni, np.arange(N)), "kernel assumes nodes_ind == arange(N)"

    xh = np.concatenate([x, h, h[:, -1:]], axis=-1)           # [N, 130] (last col 2x)
    xh_sb = _f32(xh.reshape(KT, 128, CI2).transpose(1, 0, 2).reshape(128, KT * CI2))
    wfs, bs = {}, {}
    # xh2 feature order for gate c: [h2(0..63), x(0..63), x(64)]
    perm_c = list(range(IN + 1, CI)) + list(range(0, IN)) + [IN]
    for g, W, b in (("r", W_r, b_r), ("u", W_u, b_u), ("c", W_c, b_c)):
        Wt = np.asarray(W, np.float32).transpose(1, 0, 2)     # [129(i), 16, 64]
        if g == "c":
            Wt = Wt[perm_c]
        Wim = Wt.reshape(CI * QD, IN)
        Wp = np.concatenate(
            [Wim, np.zeros((RPAD - CI * QD, IN), np.float32)], axis=0)
        wfs[g] = Wp.reshape(RT, 128, IN)
        bs[g] = np.asarray(b, np.float32)

    wfru = _f32(np.concatenate([wfs["r"], wfs["u"]], axis=2)
                .transpose(1, 0, 2).reshape(128, RT * 2 * IN))
    wfc = _f32(wfs["c"].transpose(1, 0, 2).reshape(128, RT * IN))
    bru = _f32(np.concatenate([bs["r"], bs["u"]], axis=1))
    bc = _f32(bs["c"])

    sel = np.zeros((128, 16 * 128), np.float32)
    for t in range(16):
        for p in range(128):
            sel[8 * t + p // 16, t * 128 + p] = 1.0
    sel16 = np.zeros((2, 128), np.float32)
    sel16[0, 0:16] = 1.0

    in_maps = []
    for c in range(NCORES):
        s = slice(c * NS, (c + 1) * NS)
        qTc = _f32(q[s].T)                                    # [16, 512]
        # adjT SBUF layout [p, (half, ktile, n)]: adjT[k, n] = adj[s][n, k]
        a = adj[s].T.reshape(KT, 128, 2, NH)                  # [t, p, half, n]
        adjT_sb = _f32(a.transpose(1, 2, 0, 3).reshape(128, 2 * KT * NH))
        in_maps.append({
            "adjT": adjT_sb,
            "xh": xh_sb,
            "qT": qTc,
            "qrep": np.ascontiguousarray(np.tile(qTc, (2 * (128 // QD), 1)).reshape(2, 128, NS).transpose(1, 0, 2).reshape(128, 2 * NS)),
            "xT": _f32(np.concatenate([x[s].T, np.zeros((1, NS), np.float32)], axis=0)),
            "hT": _f32(h[s].T),
            "wfru": wfru, "wfc": wfc, "bru": bru, "bc": bc,
            "sel": sel, "sel16": sel16,
        })
    return in_maps


def kernel(**inputs):
    from concourse.bass_utils import run_bass_kernel_spmd

    if "nc" not in _CACHE:
        _CACHE["nc"] = build_nc()
    nc = _CACHE["nc"]
    in_maps = prep_in_maps(**inputs)
    res = run_bass_kernel_spmd(nc, in_maps, core_ids=list(range(NCORES)))
    out = np.empty((N, IN), np.float32)
    for c in range(NCORES):
        out[c * NS:(c + 1) * NS, :] = res.results[c]["out"].T
    return out
